# revision 37
# baseline (speedup 1.0000x reference)
"""BinaryTreeLSTM on 8 Trainium2 NeuronCores.

Data-parallel over the leaf batch: core d owns leaves [1024d, 1024d+1024)
in BIT-REVERSED order and folds its subtree feature-major through the leaf
projection plus 2 merge levels (1024 -> 256 nodes); the 8x256 per-core
subtree roots are combined on the host for the remaining (tiny, serial,
latency-bound) top-of-tree levels.

Bit-reversal makes every level's left children land at free columns [0:B]
and right children at [B:2B], so all levels use identical feature-major
compute: state is [128 partitions = m-features, 2 chunks, nodes], weights
are the stationary matmul operand (bf16 -> fast weight load), h streams as
the moving operand (f32r, single-pass PE), and child reads are contiguous
slices. No transposes, no SBUF-to-SBUF gathers, no node-major regime.

Bias handling: bx is folded into the leaf matmul via an augmented ones-row
in the embedding chunk / bx-row in the Wx chunk; the internal-node pad
projection px is host-precomputed and applied via the ACT per-partition
bias (wide levels) or a rank-1 PE pass (narrow levels, prefetchable).
"""

import numpy as np

IN_DIM = 300
MEM_DIM = 256
N_LEAVES = 8192
N_CORES = 8
LPC = N_LEAVES // N_CORES  # 1024 leaves per core
B_STOP = 256               # per-core nodes returned to the host
GL = 256                   # leaf/level node-chunk size

# 5-gate order [u, i, lf, rf, o]; lf and rf share the fx slice of px
_PX5SRC = [0, 256, 512, 512, 768]

_CACHE = {}


def _bitrev_perm(n):
    bits = n.bit_length() - 1
    p = np.arange(n)
    r = np.zeros(n, dtype=np.int64)
    for b in range(bits):
        r |= ((p >> b) & 1) << (bits - 1 - b)
    return r


def _build():
    import concourse.bacc as bacc
    import concourse.mybir as mybir
    import concourse.tile as tile

    f32 = mybir.dt.float32
    f32r = mybir.dt.float32r
    f16 = mybir.dt.float16
    AF = mybir.ActivationFunctionType

    nc = bacc.Bacc("TRN2", target_bir_lowering=False, debug=False,
                   num_devices=N_CORES)

    # k-chunked inputs (separate tensors => DMA/dependency granularity)
    embsT = [nc.dram_tensor(f"embsT{k}", [128, LPC], f16,
                            kind="ExternalInput").ap() for k in range(3)]
    WxT = [nc.dram_tensor(f"WxT{k}", [128, 1024], f16,
                          kind="ExternalInput").ap() for k in range(3)]
    WlT = nc.dram_tensor("WlT", [128, 2 * 1280], f16, kind="ExternalInput").ap()
    WrT = nc.dram_tensor("WrT", [128, 2 * 1280], f16, kind="ExternalInput").ap()
    px5fm = nc.dram_tensor("px5fm", [128, 10], f32, kind="ExternalInput").ap()
    px5r = nc.dram_tensor("px5r", [1, 1280], f16, kind="ExternalInput").ap()
    out = nc.dram_tensor("out", [256, 2 * B_STOP], f16, kind="ExternalOutput").ap()

    with tile.TileContext(nc) as tc:
        with (
            tc.tile_pool(name="const", bufs=1) as const,
            tc.tile_pool(name="state", bufs=1) as state,
            tc.tile_pool(name="gates", bufs=2) as gates,
            tc.tile_pool(name="psum", bufs=1, space="PSUM") as psum,
        ):
            v2 = lambda t: t.rearrange("p (c n) -> p c n", c=2)

            # HAM warm-up source: memset, no DMA dependency, scheduled at
            # the very front so dummy matmuls can warm the PE clock gate
            # while input DMAs stream
            warm_sb = const.tile([128, 1024], f16, tag="warm")
            warm_ps = psum.tile([128, 512], f32, tag="u", bufs=2, name="warm")
            with tc.high_priority():
                nc.vector.memset(warm_sb[:, :], 1.0)
                for wi in range(7):
                    nc.tensor.matmul(warm_ps[:, :], warm_sb[:, 0:128],
                                     warm_sb[:, 0:512],
                                     start=(wi == 0), stop=(wi == 6))

            # ---- input DMAs: leaf tensors first, spread across queues ----
            WxT_sb = [const.tile([128, 1024], f16, name=f"wx{k}",
                             tag=f"wx{k}") for k in range(3)]
            embsT_sb = [const.tile([128, LPC], f16, name=f"em{k}",
                        tag=f"em{k}") for k in range(3)]
            # first-needed-first per queue; embsT ships as 512-col halves
            # so the first leaf chunk's inputs land early
            nc.scalar.dma_start(WxT_sb[0][:, 0:512], WxT[0][:, 0:512])
            nc.sync.dma_start(WxT_sb[1][:, :], WxT[1][:, :])
            nc.scalar.dma_start(WxT_sb[0][:, 512:1024], WxT[0][:, 512:1024])
            for k in range(3):  # chunk-A halves (host-packed [q0|q2|q1|q3])
                nc.gpsimd.dma_start(embsT_sb[k][:, 0:512], embsT[k][:, 0:512])
            nc.scalar.dma_start(WxT_sb[2][:, :], WxT[2][:, :])
            for k in range(3):
                nc.sync.dma_start(embsT_sb[k][:, 512:1024],
                                  embsT[k][:, 512:1024])
            WlT_sb = const.tile([128, 2 * 1280], f16, tag="wl")
            WrT_sb = const.tile([128, 2 * 1280], f16, tag="wr")
            px5fm_sb = const.tile([128, 10], f32, tag="pxf")
            px5r_sb = const.tile([1, 1280], f16, tag="pxr")
            nc.scalar.dma_start(WlT_sb[:, :], WlT[:, :])
            nc.gpsimd.dma_start(WrT_sb[:, :], WrT[:, :])
            nc.sync.dma_start(px5fm_sb[:, :], px5fm[:, :])
            nc.sync.dma_start(px5r_sb[:, :], px5r[:, :])

            ones_sb = warm_sb  # all-ones f16, used by the rank-1 px pass
            GATE_FNS = [AF.Tanh, AF.Sigmoid, AF.Sigmoid, AF.Sigmoid, AF.Sigmoid]
            GTAG = ["u", "i", "lf", "rf", "o"]

            # ---- leaf phase: 1024 leaves -> per-chunk c/h tiles ----
            # chunk A covers leaf positions {0:256, 512:768} (packed), chunk
            # B covers {256:512, 768:1024}: exactly the children of L0's two
            # node-chunks, so each L0 chunk depends on one leaf tile only.
            cL = [state.tile([128, 2 * 512], f16, name=f"c_leaf{a}",
                             tag=f"c_leaf{a}") for a in range(2)]
            hL = [state.tile([128, 2 * 512], f16, name=f"h_leaf{a}",
                             tag=f"h_leaf{a}") for a in range(2)]
            KR = [128, 128, 45]  # rows per k-chunk (chunk 2: 44 data + bias)
            LEAF_G = (("u", 0, AF.Tanh), ("i", 1, AF.Sigmoid),
                      ("o", 3, AF.Sigmoid))

            with nc.named_scope("leaf"):
                for sg in range(2):
                    ps = {}
                    for gname, gm, fn in LEAF_G:
                        for half in range(2):
                            ps[gname, half] = psum.tile(
                                [128, 512], f32, tag=gname,
                                name=f"ps_{gname}{sg}_{half}", bufs=2)
                    if sg == 0:
                        mm_order = [(ki, gname, gm, half) for ki in range(3)
                                    for gname, gm, fn in LEAF_G
                                    for half in range(2)]
                    else:
                        mm_order = [(ki, gname, gm, half)
                                    for gname, gm, fn in LEAF_G
                                    for half in range(2) for ki in range(3)]
                    for ki, gname, gm, half in mm_order:
                        m = gm * 2 + half
                        nc.tensor.matmul(
                            ps[gname, half][:, :],
                            WxT_sb[ki][0:KR[ki], m * 128:(m + 1) * 128],
                            embsT_sb[ki][0:KR[ki],
                                         sg * 512:(sg + 1) * 512],
                            start=(ki == 0), stop=(ki == 2))
                    sb = {}
                    tht = gates.tile([128, 2 * 512], f16, tag="th",
                                     name=f"th{sg}")
                    cs = v2(cL[sg])
                    for gname, gm, fn in LEAF_G:
                        t = gates.tile([128, 2 * 512], f16, tag=gname,
                                       name=f"g_{gname}{sg}")
                        for half in range(2):
                            nc.scalar.activation(
                                t[:, half * 512:(half + 1) * 512],
                                ps[gname, half][:, :], fn)
                        sb[gname] = t
                        if gname == "i":
                            nc.vector.tensor_mul(cs, v2(sb["i"]), v2(sb["u"]))
                            nc.scalar.activation(v2(tht), cs, AF.Tanh)
                    nc.vector.tensor_mul(v2(hL[sg]), v2(sb["o"]), v2(tht))

            # ---- merge level: one node-chunk of 256 parents ----
            # h_pair/c_pair: per-side [128, 2, 256] sources for this chunk
            def fm_chunk(h_pair, c_pair, lvl, ci, last):
                sfx = f"{lvl}_{ci}"
                h_n = state.tile([128, 2 * 256], f16, name=f"h{sfx}",
                                 tag=f"h{sfx}")
                c_n = state.tile([128, 2 * 256], f16, name=f"c{sfx}",
                                 tag=f"c{sfx}")
                x1 = gates.tile([128, 2 * 256], f16, tag="x1", name=f"x1{sfx}")
                x2 = gates.tile([128, 2 * 256], f16, tag="x2", name=f"x2{sfx}")
                x3 = gates.tile([128, 2 * 256], f16, tag="x3", name=f"x3{sfx}")
                s1 = gates.tile([128, 2 * 256], f16, tag="s1", name=f"s1{sfx}")
                tht = gates.tile([128, 2 * 256], f16, tag="th", name=f"th{sfx}")
                sb = {}

                def gate(gi):
                    g = gates.tile([128, 2 * 256], f16, tag=GTAG[gi],
                                   name=f"g_{GTAG[gi]}{sfx}")
                    for half in range(2):
                        m = gi * 2 + half
                        t = psum.tile([128, 256], f32, tag=GTAG[gi],
                                      name=f"ps{GTAG[gi]}{sfx}_{half}",
                                      bufs=2 if gi in (0, 1, 4) else 1)
                        for ki in range(4):
                            side, kc = ki // 2, ki % 2
                            W = WlT_sb if side == 0 else WrT_sb
                            nc.tensor.matmul(
                                t[:, :],
                                W[:, kc * 1280 + m * 128:
                                  kc * 1280 + (m + 1) * 128],
                                h_pair[side][:, kc, :],
                                start=(ki == 0), stop=(ki == 3))
                        nc.scalar.activation(
                            g[:, half * 256:(half + 1) * 256], t[:, :],
                            GATE_FNS[gi],
                            bias=px5fm_sb[:, gi * 2 + half:gi * 2 + half + 1])
                    sb[gi] = g

                # emission order = scheduler priority: the c-chain lands
                # between gate groups so it runs under the o-gate matmuls
                gate(0)
                gate(1)
                nc.vector.tensor_mul(v2(x1), v2(sb[1]), v2(sb[0]))
                gate(2)
                nc.vector.tensor_mul(v2(x2), v2(sb[2]), c_pair[0])
                nc.vector.tensor_add(v2(s1), v2(x1), v2(x2))
                gate(3)
                nc.vector.tensor_mul(v2(x3), v2(sb[3]), c_pair[1])
                cs = v2(c_n)
                nc.vector.tensor_add(cs, v2(s1), v2(x3))
                nc.scalar.activation(v2(tht), cs, AF.Tanh)
                gate(4)
                nc.vector.tensor_mul(v2(h_n), v2(sb[4]), v2(tht))
                return h_n, c_n

            # L0: chunk ci reads leaf tile ci (left cols 0:256, right 256:512)
            h1, c1 = [], []
            with nc.named_scope("L0_B512"):
                for ci in range(2):
                    hp = v2(hL[ci])
                    cp = v2(cL[ci])
                    hn, cn = fm_chunk(
                        [hp[:, :, 0:256], hp[:, :, 256:512]],
                        [cp[:, :, 0:256], cp[:, :, 256:512]],
                        0, ci, False)
                    h1.append(hn)
                    c1.append(cn)
            # L1: left = L0 chunk 0, right = L0 chunk 1
            with nc.named_scope("L1_B256"):
                h, c = fm_chunk([v2(h1[0]), v2(h1[1])],
                                [v2(c1[0]), v2(c1[1])], 1, 0, True)

            nc.sync.dma_start(out[0:128, :], c[:, :])
            nc.scalar.dma_start(out[128:256, :], h[:, :])

    nc.compile()
    return nc


def _get_nc():
    if "nc" not in _CACHE:
        _CACHE["nc"] = _build()
    return _CACHE["nc"]


def kernel(embs, Wx, bx, Wl, Wr, emb_table, _trace=False, _trace_kwargs=None):
    from concourse.bass_utils import run_bass_kernel_spmd

    embs = np.asarray(embs, dtype=np.float32)
    Wx = np.asarray(Wx, dtype=np.float32)
    bx = np.asarray(bx, dtype=np.float32)
    Wl = np.asarray(Wl, dtype=np.float32)
    Wr = np.asarray(Wr, dtype=np.float32)
    emb_table = np.asarray(emb_table, dtype=np.float32)

    WxT = np.ascontiguousarray(Wx.T)                      # [300, 1024]
    WlT = np.ascontiguousarray(Wl.T)                      # [256, 1280]
    WrT = np.ascontiguousarray(Wr.T)

    # Wx chunks with bx folded in as an extra contraction row (row 44 of
    # chunk 2, matching the ones-row in the embedding chunk)
    WxT_ch = []
    for k in range(2):
        WxT_ch.append(np.ascontiguousarray(
            WxT[128 * k:128 * (k + 1)].astype(np.float16)))
    w2 = np.zeros((128, 1024), dtype=np.float16)
    w2[0:44] = WxT[256:300].astype(np.float16)
    w2[44] = bx.astype(np.float16)
    WxT_ch.append(w2)

    # weight images [128, 2*1280] (k-chunks side by side), fp16
    WlT_img = np.ascontiguousarray(
        np.concatenate([WlT[0:128], WlT[128:256]], axis=1).astype(np.float16))
    WrT_img = np.ascontiguousarray(
        np.concatenate([WrT[0:128], WrT[128:256]], axis=1).astype(np.float16))

    # pad-node x-projection, expanded to the 5-gate layout
    px = emb_table[-1] @ WxT + bx                          # [1024]
    px5 = np.concatenate([px[s:s + 256] for s in _PX5SRC]) # [1280]
    px5r = np.ascontiguousarray(px5.reshape(1, 1280).astype(np.float16))
    px5fm = np.ascontiguousarray(px5.reshape(10, 128).T)   # [128, 10]

    perm = _bitrev_perm(LPC)
    qorder = np.r_[0:256, 512:768, 256:512, 768:1024]
    perm = perm[qorder]  # leaf chunk A = L0-chunk-0's children, packed
    in_maps = []
    for d in range(N_CORES):
        shard = embs[d * LPC:(d + 1) * LPC][perm].T.astype(np.float16)
        e2 = np.zeros((128, LPC), dtype=np.float16)
        e2[0:44] = shard[256:300]
        e2[44] = 1.0
        in_maps.append({
            "embsT0": np.ascontiguousarray(shard[0:128]),
            "embsT1": np.ascontiguousarray(shard[128:256]),
            "embsT2": e2,
            "WxT0": WxT_ch[0], "WxT1": WxT_ch[1], "WxT2": WxT_ch[2],
            "WlT": WlT_img, "WrT": WrT_img,
            "px5fm": px5fm, "px5r": px5r,
        })

    nc = _get_nc()
    res = run_bass_kernel_spmd(nc, in_maps, list(range(N_CORES)),
                               trace=_trace, **(_trace_kwargs or {}))
    _CACHE["last_result"] = res

    # ---- unshard: un-bit-reverse, then fold the remaining levels ----
    rperm = _bitrev_perm(B_STOP)  # position p holds node rperm[p]
    cs, hs = [], []
    for d in range(N_CORES):
        o = np.asarray(res.results[d]["out"], dtype=np.float32)
        cf = o[0:128].reshape(128, 2, B_STOP)
        hf = o[128:256].reshape(128, 2, B_STOP)
        c_nm = np.concatenate([cf[:, 0, :], cf[:, 1, :]], axis=0).T  # [B,256]
        h_nm = np.concatenate([hf[:, 0, :], hf[:, 1, :]], axis=0).T
        inv = np.empty(B_STOP, dtype=np.int64)
        inv[rperm] = np.arange(B_STOP)
        cs.append(c_nm[inv])   # node order
        hs.append(h_nm[inv])
    c = np.concatenate(cs, axis=0)  # [512, 256]
    h = np.concatenate(hs, axis=0)
    m = MEM_DIM

    def sig(x):
        return 1.0 / (1.0 + np.exp(-x))

    while c.shape[0] > 1:
        lg = h[0::2] @ WlT
        rg = h[1::2] @ WrT
        u = np.tanh(px[0:m] + lg[:, 0:m] + rg[:, 0:m])
        i = sig(px[m:2 * m] + lg[:, m:2 * m] + rg[:, m:2 * m])
        lf = sig(px[2 * m:3 * m] + lg[:, 2 * m:3 * m] + rg[:, 2 * m:3 * m])
        rf = sig(px[2 * m:3 * m] + lg[:, 3 * m:4 * m] + rg[:, 3 * m:4 * m])
        o = sig(px[3 * m:4 * m] + lg[:, 4 * m:5 * m] + rg[:, 4 * m:5 * m])
        c = i * u + lf * c[0::2] + rf * c[1::2]
        h = o * np.tanh(c)
    return np.stack([c, h]).astype(np.float32)


# revision 38
# speedup vs baseline: 1.0923x; 1.0923x over previous
"""BinaryTreeLSTM on 8 Trainium2 NeuronCores.

Data-parallel over the leaf batch: core d owns leaves [1024d, 1024d+1024)
in BIT-REVERSED order and folds its subtree feature-major through the leaf
projection plus 2 merge levels (1024 -> 256 nodes); the 8x256 per-core
subtree roots are combined on the host for the remaining (tiny, serial,
latency-bound) top-of-tree levels.

Bit-reversal makes every level's left children land at free columns [0:B]
and right children at [B:2B], so all levels use identical feature-major
compute: state is [128 partitions = m-features, 2 chunks, nodes], weights
are the stationary matmul operand (bf16 -> fast weight load), h streams as
the moving operand (f32r, single-pass PE), and child reads are contiguous
slices. No transposes, no SBUF-to-SBUF gathers, no node-major regime.

Bias handling: bx is folded into the leaf matmul via an augmented ones-row
in the embedding chunk / bx-row in the Wx chunk; the internal-node pad
projection px is host-precomputed and applied via the ACT per-partition
bias (wide levels) or a rank-1 PE pass (narrow levels, prefetchable).
"""

import numpy as np

IN_DIM = 300
MEM_DIM = 256
N_LEAVES = 8192
N_CORES = 8
LPC = N_LEAVES // N_CORES  # 1024 leaves per core
B_STOP = 256               # per-core nodes returned to the host
GL = 256                   # leaf/level node-chunk size

# 5-gate order [u, i, lf, rf, o]; lf and rf share the fx slice of px
_PX5SRC = [0, 256, 512, 512, 768]

_CACHE = {}


def _bitrev_perm(n):
    bits = n.bit_length() - 1
    p = np.arange(n)
    r = np.zeros(n, dtype=np.int64)
    for b in range(bits):
        r |= ((p >> b) & 1) << (bits - 1 - b)
    return r


def _build():
    import concourse.bacc as bacc
    import concourse.mybir as mybir
    import concourse.tile as tile

    f32 = mybir.dt.float32
    f32r = mybir.dt.float32r
    f16 = mybir.dt.float16
    AF = mybir.ActivationFunctionType

    nc = bacc.Bacc("TRN2", target_bir_lowering=False, debug=False,
                   num_devices=N_CORES)

    # k-chunked inputs (separate tensors => DMA/dependency granularity)
    embsT = [nc.dram_tensor(f"embsT{k}", [128, LPC], f16,
                            kind="ExternalInput").ap() for k in range(3)]
    WxT = [nc.dram_tensor(f"WxT{k}", [128, 1024], f16,
                          kind="ExternalInput").ap() for k in range(3)]
    WlT = nc.dram_tensor("WlT", [128, 2 * 1280], f16, kind="ExternalInput").ap()
    WrT = nc.dram_tensor("WrT", [128, 2 * 1280], f16, kind="ExternalInput").ap()
    px5fm = nc.dram_tensor("px5fm", [128, 10], f32, kind="ExternalInput").ap()
    px5r = nc.dram_tensor("px5r", [1, 1280], f16, kind="ExternalInput").ap()
    out = nc.dram_tensor("out", [256, 2 * B_STOP], f16, kind="ExternalOutput").ap()

    with tile.TileContext(nc) as tc:
        with (
            tc.tile_pool(name="const", bufs=1) as const,
            tc.tile_pool(name="state", bufs=1) as state,
            tc.tile_pool(name="gates", bufs=2) as gates,
            tc.tile_pool(name="psum", bufs=1, space="PSUM") as psum,
        ):
            v2 = lambda t: t.rearrange("p (c n) -> p c n", c=2)

            # HAM warm-up source: memset, no DMA dependency, scheduled at
            # the very front so dummy matmuls can warm the PE clock gate
            # while input DMAs stream
            warm_sb = const.tile([128, 1024], f16, tag="warm")
            warm_ps = psum.tile([128, 512], f32, tag="u", bufs=2, name="warm")
            with tc.high_priority():
                nc.vector.memset(warm_sb[:, :], 1.0)
                for wi in range(7):
                    nc.tensor.matmul(warm_ps[:, :], warm_sb[:, 0:128],
                                     warm_sb[:, 0:512],
                                     start=(wi == 0), stop=(wi == 6))

            # ---- input DMAs: leaf tensors first, spread across queues ----
            WxT_sb = [const.tile([128, 1024], f16, name=f"wx{k}",
                             tag=f"wx{k}") for k in range(3)]
            embsT_sb = [const.tile([128, LPC], f16, name=f"em{k}",
                        tag=f"em{k}") for k in range(3)]
            # first-needed-first per queue; embsT ships as 512-col halves
            # so the first leaf chunk's inputs land early
            nc.scalar.dma_start(WxT_sb[0][:, 0:512], WxT[0][:, 0:512])
            nc.sync.dma_start(WxT_sb[1][:, :], WxT[1][:, :])
            nc.scalar.dma_start(WxT_sb[0][:, 512:1024], WxT[0][:, 512:1024])
            for k in range(3):  # chunk-A halves (host-packed [q0|q2|q1|q3])
                nc.gpsimd.dma_start(embsT_sb[k][:, 0:512], embsT[k][:, 0:512])
            nc.scalar.dma_start(WxT_sb[2][:, :], WxT[2][:, :])
            for k in range(3):
                nc.sync.dma_start(embsT_sb[k][:, 512:1024],
                                  embsT[k][:, 512:1024])
            WlT_sb = const.tile([128, 2 * 1280], f16, tag="wl")
            WrT_sb = const.tile([128, 2 * 1280], f16, tag="wr")
            px5fm_sb = const.tile([128, 10], f32, tag="pxf")
            px5r_sb = const.tile([1, 1280], f16, tag="pxr")
            nc.scalar.dma_start(WlT_sb[:, :], WlT[:, :])
            nc.gpsimd.dma_start(WrT_sb[:, :], WrT[:, :])
            nc.sync.dma_start(px5fm_sb[:, :], px5fm[:, :])
            nc.sync.dma_start(px5r_sb[:, :], px5r[:, :])

            ones_sb = warm_sb  # all-ones f16, used by the rank-1 px pass
            GATE_FNS = [AF.Tanh, AF.Sigmoid, AF.Sigmoid, AF.Sigmoid, AF.Sigmoid]
            GTAG = ["u", "i", "lf", "rf", "o"]

            # ---- leaf phase: 1024 leaves -> per-chunk c/h tiles ----
            # chunk A covers leaf positions {0:256, 512:768} (packed), chunk
            # B covers {256:512, 768:1024}: exactly the children of L0's two
            # node-chunks, so each L0 chunk depends on one leaf tile only.
            cL = [state.tile([128, 2 * 512], f16, name=f"c_leaf{a}",
                             tag=f"c_leaf{a}") for a in range(2)]
            hL = [state.tile([128, 2 * 512], f16, name=f"h_leaf{a}",
                             tag=f"h_leaf{a}") for a in range(2)]
            KR = [128, 128, 45]  # rows per k-chunk (chunk 2: 44 data + bias)
            LEAF_G = (("u", 0, AF.Tanh), ("i", 1, AF.Sigmoid),
                      ("o", 3, AF.Sigmoid))

            with nc.named_scope("leaf"):
                for sg in range(2):
                    ps = {}
                    for gname, gm, fn in LEAF_G:
                        for half in range(2):
                            ps[gname, half] = psum.tile(
                                [128, 512], f32, tag=gname,
                                name=f"ps_{gname}{sg}_{half}", bufs=2)
                    if sg == 0:
                        mm_order = [(ki, gname, gm, half) for ki in range(3)
                                    for gname, gm, fn in LEAF_G
                                    for half in range(2)]
                    else:
                        mm_order = [(ki, gname, gm, half)
                                    for gname, gm, fn in LEAF_G
                                    for half in range(2) for ki in range(3)]
                    for ki, gname, gm, half in mm_order:
                        m = gm * 2 + half
                        nc.tensor.matmul(
                            ps[gname, half][:, :],
                            WxT_sb[ki][0:KR[ki], m * 128:(m + 1) * 128],
                            embsT_sb[ki][0:KR[ki],
                                         sg * 512:(sg + 1) * 512],
                            start=(ki == 0), stop=(ki == 2))
                    sb = {}
                    for gname, gm, fn in LEAF_G:
                        t = gates.tile([128, 2 * 512], f16, tag=gname,
                                       name=f"g_{gname}{sg}")
                        for half in range(2):
                            nc.scalar.activation(
                                t[:, half * 512:(half + 1) * 512],
                                ps[gname, half][:, :], fn)
                        sb[gname] = t
                    tht = gates.tile([128, 2 * 512], f16, tag="th",
                                     name=f"th{sg}")
                    cs = v2(cL[sg])
                    nc.vector.tensor_mul(cs, v2(sb["i"]), v2(sb["u"]))
                    nc.scalar.activation(v2(tht), cs, AF.Tanh)
                    nc.vector.tensor_mul(v2(hL[sg]), v2(sb["o"]), v2(tht))

            # ---- merge level: one node-chunk of 256 parents ----
            # h_pair/c_pair: per-side [128, 2, 256] sources for this chunk
            def fm_chunk(h_pair, c_pair, lvl, ci, last):
                sfx = f"{lvl}_{ci}"
                h_n = state.tile([128, 2 * 256], f16, name=f"h{sfx}",
                                 tag=f"h{sfx}")
                c_n = state.tile([128, 2 * 256], f16, name=f"c{sfx}",
                                 tag=f"c{sfx}")
                x1 = gates.tile([128, 2 * 256], f16, tag="x1", name=f"x1{sfx}")
                x2 = gates.tile([128, 2 * 256], f16, tag="x2", name=f"x2{sfx}")
                x3 = gates.tile([128, 2 * 256], f16, tag="x3", name=f"x3{sfx}")
                s1 = gates.tile([128, 2 * 256], f16, tag="s1", name=f"s1{sfx}")
                tht = gates.tile([128, 2 * 256], f16, tag="th", name=f"th{sfx}")
                sb = {}

                def gate(gi):
                    g = gates.tile([128, 2 * 256], f16, tag=GTAG[gi],
                                   name=f"g_{GTAG[gi]}{sfx}")
                    for half in range(2):
                        m = gi * 2 + half
                        t = psum.tile([128, 256], f32, tag=GTAG[gi],
                                      name=f"ps{GTAG[gi]}{sfx}_{half}",
                                      bufs=2 if gi in (0, 1, 4) else 1)
                        for ki in range(4):
                            side, kc = ki // 2, ki % 2
                            W = WlT_sb if side == 0 else WrT_sb
                            nc.tensor.matmul(
                                t[:, :],
                                W[:, kc * 1280 + m * 128:
                                  kc * 1280 + (m + 1) * 128],
                                h_pair[side][:, kc, :],
                                start=(ki == 0), stop=(ki == 3))
                        nc.scalar.activation(
                            g[:, half * 256:(half + 1) * 256], t[:, :],
                            GATE_FNS[gi],
                            bias=px5fm_sb[:, gi * 2 + half:gi * 2 + half + 1])
                    sb[gi] = g

                # emission order = scheduler priority: the c-chain lands
                # between gate groups so it runs under the o-gate matmuls
                gate(0)
                gate(1)
                nc.vector.tensor_mul(v2(x1), v2(sb[1]), v2(sb[0]))
                gate(2)
                nc.vector.tensor_mul(v2(x2), v2(sb[2]), c_pair[0])
                nc.vector.tensor_add(v2(s1), v2(x1), v2(x2))
                gate(3)
                nc.vector.tensor_mul(v2(x3), v2(sb[3]), c_pair[1])
                cs = v2(c_n)
                nc.vector.tensor_add(cs, v2(s1), v2(x3))
                nc.scalar.activation(v2(tht), cs, AF.Tanh)
                gate(4)
                nc.vector.tensor_mul(v2(h_n), v2(sb[4]), v2(tht))
                return h_n, c_n

            # L0: chunk ci reads leaf tile ci (left cols 0:256, right 256:512)
            h1, c1 = [], []
            with nc.named_scope("L0_B512"):
                for ci in range(2):
                    hp = v2(hL[ci])
                    cp = v2(cL[ci])
                    hn, cn = fm_chunk(
                        [hp[:, :, 0:256], hp[:, :, 256:512]],
                        [cp[:, :, 0:256], cp[:, :, 256:512]],
                        0, ci, False)
                    h1.append(hn)
                    c1.append(cn)
            # L1: left = L0 chunk 0, right = L0 chunk 1
            with nc.named_scope("L1_B256"):
                h, c = fm_chunk([v2(h1[0]), v2(h1[1])],
                                [v2(c1[0]), v2(c1[1])], 1, 0, True)

            nc.sync.dma_start(out[0:128, :], c[:, :])
            nc.scalar.dma_start(out[128:256, :], h[:, :])

    nc.compile()
    return nc


def _get_nc():
    if "nc" not in _CACHE:
        _CACHE["nc"] = _build()
    return _CACHE["nc"]


def kernel(embs, Wx, bx, Wl, Wr, emb_table, _trace=False, _trace_kwargs=None):
    from concourse.bass_utils import run_bass_kernel_spmd

    embs = np.asarray(embs, dtype=np.float32)
    Wx = np.asarray(Wx, dtype=np.float32)
    bx = np.asarray(bx, dtype=np.float32)
    Wl = np.asarray(Wl, dtype=np.float32)
    Wr = np.asarray(Wr, dtype=np.float32)
    emb_table = np.asarray(emb_table, dtype=np.float32)

    WxT = np.ascontiguousarray(Wx.T)                      # [300, 1024]
    WlT = np.ascontiguousarray(Wl.T)                      # [256, 1280]
    WrT = np.ascontiguousarray(Wr.T)

    # Wx chunks with bx folded in as an extra contraction row (row 44 of
    # chunk 2, matching the ones-row in the embedding chunk)
    WxT_ch = []
    for k in range(2):
        WxT_ch.append(np.ascontiguousarray(
            WxT[128 * k:128 * (k + 1)].astype(np.float16)))
    w2 = np.zeros((128, 1024), dtype=np.float16)
    w2[0:44] = WxT[256:300].astype(np.float16)
    w2[44] = bx.astype(np.float16)
    WxT_ch.append(w2)

    # weight images [128, 2*1280] (k-chunks side by side), fp16
    WlT_img = np.ascontiguousarray(
        np.concatenate([WlT[0:128], WlT[128:256]], axis=1).astype(np.float16))
    WrT_img = np.ascontiguousarray(
        np.concatenate([WrT[0:128], WrT[128:256]], axis=1).astype(np.float16))

    # pad-node x-projection, expanded to the 5-gate layout
    px = emb_table[-1] @ WxT + bx                          # [1024]
    px5 = np.concatenate([px[s:s + 256] for s in _PX5SRC]) # [1280]
    px5r = np.ascontiguousarray(px5.reshape(1, 1280).astype(np.float16))
    px5fm = np.ascontiguousarray(px5.reshape(10, 128).T)   # [128, 10]

    perm = _bitrev_perm(LPC)
    qorder = np.r_[0:256, 512:768, 256:512, 768:1024]
    perm = perm[qorder]  # leaf chunk A = L0-chunk-0's children, packed
    in_maps = []
    for d in range(N_CORES):
        shard = embs[d * LPC:(d + 1) * LPC][perm].T.astype(np.float16)
        e2 = np.zeros((128, LPC), dtype=np.float16)
        e2[0:44] = shard[256:300]
        e2[44] = 1.0
        in_maps.append({
            "embsT0": np.ascontiguousarray(shard[0:128]),
            "embsT1": np.ascontiguousarray(shard[128:256]),
            "embsT2": e2,
            "WxT0": WxT_ch[0], "WxT1": WxT_ch[1], "WxT2": WxT_ch[2],
            "WlT": WlT_img, "WrT": WrT_img,
            "px5fm": px5fm, "px5r": px5r,
        })

    nc = _get_nc()
    res = run_bass_kernel_spmd(nc, in_maps, list(range(N_CORES)),
                               trace=_trace, **(_trace_kwargs or {}))
    _CACHE["last_result"] = res

    # ---- unshard: un-bit-reverse, then fold the remaining levels ----
    rperm = _bitrev_perm(B_STOP)  # position p holds node rperm[p]
    cs, hs = [], []
    for d in range(N_CORES):
        o = np.asarray(res.results[d]["out"], dtype=np.float32)
        cf = o[0:128].reshape(128, 2, B_STOP)
        hf = o[128:256].reshape(128, 2, B_STOP)
        c_nm = np.concatenate([cf[:, 0, :], cf[:, 1, :]], axis=0).T  # [B,256]
        h_nm = np.concatenate([hf[:, 0, :], hf[:, 1, :]], axis=0).T
        inv = np.empty(B_STOP, dtype=np.int64)
        inv[rperm] = np.arange(B_STOP)
        cs.append(c_nm[inv])   # node order
        hs.append(h_nm[inv])
    c = np.concatenate(cs, axis=0)  # [512, 256]
    h = np.concatenate(hs, axis=0)
    m = MEM_DIM

    def sig(x):
        return 1.0 / (1.0 + np.exp(-x))

    while c.shape[0] > 1:
        lg = h[0::2] @ WlT
        rg = h[1::2] @ WrT
        u = np.tanh(px[0:m] + lg[:, 0:m] + rg[:, 0:m])
        i = sig(px[m:2 * m] + lg[:, m:2 * m] + rg[:, m:2 * m])
        lf = sig(px[2 * m:3 * m] + lg[:, 2 * m:3 * m] + rg[:, 2 * m:3 * m])
        rf = sig(px[2 * m:3 * m] + lg[:, 3 * m:4 * m] + rg[:, 3 * m:4 * m])
        o = sig(px[3 * m:4 * m] + lg[:, 4 * m:5 * m] + rg[:, 4 * m:5 * m])
        c = i * u + lf * c[0::2] + rf * c[1::2]
        h = o * np.tanh(c)
    return np.stack([c, h]).astype(np.float32)
